# revision 5
# baseline (speedup 1.0000x reference)
"""Ernie4.5 attention block (T=2048, H=4096, 32 Q heads / 8 KV heads, rope,
causal, o_proj) on 8 Trainium2 NeuronCores.

Sharding: tensor-parallel by head. Each core computes QKV^T for its 4 Q heads
+ 1 KV head (column-sharded w_qkv), runs attention for those heads, then the
attention outputs (kept transposed, head-dim on partitions) are AllGather'd
and each core computes a 512-column shard of o_proj (column-sharded w_o).
The host only does layout work: transpose/permute/shard inputs, concatenate
output shards.

Device numerics: matmuls in float32r (QKV, QK^T, PV, Z) and bfloat16
(o_proj); softmax in fp32 without max-subtraction (scores are O(6), exp is
safe in fp32); denominator Z computed on the PE with a ones-vector matmul and
divided out after PV.
"""

import numpy as np

import concourse.bacc as bacc
import concourse.mybir as mybir
import concourse.tile as tile
from concourse.bass_utils import run_bass_kernel_spmd
from concourse.masks import make_identity

T = 2048
HID = 4096
NH = 32
NKV = 8
HD = 128
THETA = 500000.0
SCALE = HD ** -0.5
N_CORES = 8
HPC = NH // N_CORES          # q heads per core
KS = HID // 128              # 32 k-subtiles
ACH = 256                    # QKV-phase t-chunk width
A_NCH = T // ACH
TCH = 512                    # attention t-chunk width
NTCH = T // TCH
SB = TCH // 128              # s-tiles per t-chunk (4)
OCH = 256                    # o_proj t-subchunk width

F32 = mybir.dt.float32
F32R = mybir.dt.float32r
BF16 = mybir.dt.bfloat16

QKV_M = HPC + 2              # m-tiles in qkv^T: 4 q heads, k, v
KROW = HPC * 128             # row offset of k block in qkvT (512)
VROW = KROW + 128            # row offset of v block (640)


def _emit_body(nc, tc, io, rep):
    hid8, wq6, c1_d, c2_d, masks_d, wo_d, outT_d = io

    with (
        tc.tile_pool(name=f"const{rep}", bufs=1) as cpool,
        tc.tile_pool(name=f"dram{rep}", bufs=1, space="DRAM") as dpool,
    ):
        # ---- constants ----
        c1 = cpool.tile([128, T], F32)
        c2 = cpool.tile([128, T], F32)
        nc.sync.dma_start(c1[:], c1_d[:])
        nc.sync.dma_start(c2[:], c2_d[:])
        masks = cpool.tile([128, SB, TCH], F32)
        nc.sync.dma_start(masks[:], masks_d.rearrange("(k p) t -> p k t", p=128))
        ident = cpool.tile([128, 128], F32)
        make_identity(nc, ident[:])
        ones_f = cpool.tile([128, 1], F32)
        nc.vector.memset(ones_f[:], 1.0)
        ones = cpool.tile([128, 1], F32R)
        nc.vector.tensor_copy(ones[:], ones_f[:])

        qkvT_dram = dpool.tile([QKV_M * 128, T], F32)

        # ---- phase A: qkvT[c, t] = w_qkvp.T @ hidden.T (column-sharded) ----
        with (
            tc.tile_pool(name=f"pa{rep}", bufs=1) as pa,
            tc.tile_pool(name=f"psa{rep}", bufs=4, space="PSUM") as psa,
        ):
            wt = []
            for m in range(QKV_M):
                w = pa.tile([128, KS, 128], F32R, tag=f"wt{m}")
                nc.sync.dma_start(w[:], wq6[m].bitcast(F32R))
                wt.append(w)
            for ci in range(A_NCH):
                ht = pa.tile([128, KS, ACH], F32R, tag="hid")
                nc.sync.dma_start(ht[:], hid8[ci].bitcast(F32R))
                for m in range(QKV_M):
                    ps = psa.tile([128, ACH], F32, tag="psA")
                    for k in range(KS):
                        nc.tensor.matmul(
                            ps[:], wt[m][:, k], ht[:, k],
                            start=(k == 0), stop=(k == KS - 1),
                        )
                    ot = pa.tile([128, ACH], F32, tag="aout", bufs=3)
                    nc.vector.tensor_copy(ot[:], ps[:])
                    nc.sync.dma_start(
                        qkvT_dram[m * 128:(m + 1) * 128, ci * ACH:(ci + 1) * ACH],
                        ot[:],
                    )

        # ---- phase B: rope, attention, allgather, o_proj ----
        with tc.tile_pool(name=f"pb{rep}", bufs=1) as pb:
            wot = pb.tile([128, KS, 512], BF16)
            nc.sync.dma_start(wot[:], wo_d[:])

            def rope(dst, src_rows):
                # dst is F32R; only the final add writes it (f32r rounding).
                raw = pb.tile([128, T], F32, tag="rraw", bufs=2)
                nc.sync.dma_start(raw[:], qkvT_dram[src_rows:src_rows + 128, :])
                sw = pb.tile([128, T], F32, tag="rswap", bufs=2)
                nc.vector.tensor_copy(sw[0:64], raw[64:128])
                nc.vector.tensor_copy(sw[64:128], raw[0:64])
                ta = pb.tile([128, T], F32, tag="ropetmp", bufs=2)
                nc.vector.tensor_tensor(ta[:], raw[:], c1[:], mybir.AluOpType.mult)
                nc.vector.tensor_tensor(sw[:], sw[:], c2[:], mybir.AluOpType.mult)
                nc.vector.tensor_tensor(dst[:], ta[:], sw[:], mybir.AluOpType.add)

            kT = pb.tile([128, T], F32R, tag="kT")
            rope(kT, KROW)

            # v: transpose [HD, T] -> 16 tiles of [s,128d]
            vraw = pb.tile([128, T], F32, tag="rraw", bufs=2)
            nc.sync.dma_start(vraw[:], qkvT_dram[VROW:VROW + 128, :])
            v_nat = pb.tile([128, T // 128, 128], F32R, tag="vnat")
            with tc.tile_pool(name=f"pst{rep}", bufs=2, space="PSUM") as pst:
                for j in range(T // 128):
                    tp = pst.tile([128, 128], F32, tag="tp")
                    nc.tensor.transpose(tp[:], vraw[:, j * 128:(j + 1) * 128], ident[:])
                    nc.vector.tensor_copy(v_nat[:, j], tp[:])

            qT = []
            for h in range(HPC):
                q = pb.tile([128, T], F32R, tag=f"qT{h}")
                rope(q, h * 128)
                qT.append(q)

            with tc.tile_pool(name=f"psb{rep}", bufs=1, space="PSUM") as psb:
                for c in range(NTCH):
                    ag_in = dpool.tile([HPC * 128, TCH], BF16, tag="agin", bufs=2)
                    nj = SB * (c + 1)
                    for h in range(HPC):
                        pv = psb.tile([128, TCH], F32, tag="pv", bufs=2)
                        zp = psb.tile([1, TCH], F32, tag="z", bufs=1)
                        for j in range(nj):
                            sc = psb.tile([128, TCH], F32, tag="sc", bufs=2)
                            nc.tensor.matmul(
                                sc[:],
                                kT[:, j * 128:(j + 1) * 128],
                                qT[h][:, c * TCH:(c + 1) * TCH],
                                start=True, stop=True,
                            )
                            e = pb.tile([128, TCH], F32R, tag="expT", bufs=3)
                            kd = j - SB * c
                            if kd >= 0:
                                # diagonal block: exp to fp32 then mask-mult
                                # (the mult rounds to f32r)
                                e0 = pb.tile([128, TCH], F32, tag="expTmp", bufs=2)
                                nc.scalar.activation(
                                    e0[:], sc[:], mybir.ActivationFunctionType.Exp,
                                    scale=SCALE,
                                )
                                nc.vector.tensor_tensor(
                                    e[:], e0[:], masks[:, kd], mybir.AluOpType.mult
                                )
                            else:
                                nc.scalar.activation(
                                    e[:], sc[:], mybir.ActivationFunctionType.Exp,
                                    scale=SCALE,
                                )
                            nc.tensor.matmul(
                                pv[:], v_nat[:, j], e[:],
                                start=(j == 0), stop=(j == nj - 1),
                            )
                            nc.tensor.matmul(
                                zp[:], ones[:], e[:],
                                start=(j == 0), stop=(j == nj - 1),
                            )
                        zr = pb.tile([1, TCH], F32, tag="zr", bufs=2)
                        nc.vector.reciprocal(zr[:], zp[:])
                        zb = pb.tile([128, TCH], F32, tag="zb", bufs=2)
                        nc.gpsimd.partition_broadcast(zb[:], zr[:])
                        at = pb.tile([128, TCH], BF16, tag="attnT", bufs=2)
                        nc.vector.tensor_tensor(
                            at[:], pv[:], zb[:], mybir.AluOpType.mult
                        )
                        nc.sync.dma_start(ag_in[h * 128:(h + 1) * 128, :], at[:])

                    ag_out = dpool.tile(
                        [NH * HD, TCH], BF16, tag="agout", bufs=2, addr_space="Shared"
                    )
                    nc.gpsimd.collective_compute(
                        "AllGather",
                        mybir.AluOpType.bypass,
                        replica_groups=[list(range(N_CORES))],
                        ins=[ag_in[:].opt()],
                        outs=[ag_out[:].opt()],
                    )

                    # o_proj for this t-chunk: outT[n, t] += w_o[:,n].T @ attnT
                    ag_re = ag_out.rearrange("(ko ki) t -> ki ko t", ki=128)
                    for s in range(TCH // OCH):
                        rt = pb.tile([128, KS, OCH], BF16, tag="agsb", bufs=2)
                        nc.sync.dma_start(
                            rt[:], ag_re[:, :, s * OCH:(s + 1) * OCH]
                        )
                        for m in range(4):
                            po = psb.tile([128, OCH], F32, tag="po", bufs=2)
                            for k in range(KS):
                                nc.tensor.matmul(
                                    po[:], wot[:, k, m * 128:(m + 1) * 128], rt[:, k],
                                    start=(k == 0), stop=(k == KS - 1),
                                )
                            oo = pb.tile([128, OCH], F32, tag="oout", bufs=2)
                            nc.vector.tensor_copy(oo[:], po[:])
                            nc.sync.dma_start(
                                outT_d[m * 128:(m + 1) * 128,
                                       c * TCH + s * OCH:c * TCH + (s + 1) * OCH],
                                oo[:],
                            )


def build_program(reps=1):
    nc = bacc.Bacc("TRN2", target_bir_lowering=False, debug=False,
                   num_devices=N_CORES)
    hid8 = nc.dram_tensor("hid8", [A_NCH, 128, KS, ACH], F32, kind="ExternalInput")
    wq6 = nc.dram_tensor("wq6", [QKV_M, 128, KS, 128], F32, kind="ExternalInput")
    c1_d = nc.dram_tensor("c1", [128, T], F32, kind="ExternalInput")
    c2_d = nc.dram_tensor("c2", [128, T], F32, kind="ExternalInput")
    masks_d = nc.dram_tensor("masks", [SB * 128, TCH], F32, kind="ExternalInput")
    wo_d = nc.dram_tensor("wo", [128, KS, 512], BF16, kind="ExternalInput")
    outT_d = nc.dram_tensor("outT", [512, T], F32, kind="ExternalOutput")
    io = (hid8, wq6, c1_d, c2_d, masks_d, wo_d, outT_d)
    with tile.TileContext(nc) as tc:
        for rep in range(reps):
            _emit_body(nc, tc, io, rep)
    nc.compile()
    return nc


def make_core_inputs(positions, hidden_states, w_qkv, w_o):
    """Host-side layout prep. Returns list of per-core input dicts."""
    positions = np.asarray(positions)
    hidden_states = np.asarray(hidden_states, dtype=np.float32)
    w_qkv = np.asarray(w_qkv, dtype=np.float32)
    w_o = np.asarray(w_o, dtype=np.float32)
    assert np.all(np.diff(positions.astype(np.int64)) > 0), (
        "kernel assumes strictly increasing positions (causal mask == index mask)"
    )

    # rope tables
    half = HD // 2
    inv_freq = 1.0 / (THETA ** (np.arange(0, half, dtype=np.float32) * 2.0 / HD))
    ang = positions.astype(np.float32)[:, None] * inv_freq[None, :]  # [T, 64]
    cosT = np.cos(ang).T.astype(np.float32)  # [64, T]
    sinT = np.sin(ang).T.astype(np.float32)
    c1 = np.ascontiguousarray(np.concatenate([cosT, cosT], axis=0))
    c2 = np.ascontiguousarray(np.concatenate([-sinT, sinT], axis=0))

    # diagonal-block causal masks
    masks = np.zeros((SB, 128, TCH), dtype=np.float32)
    s_idx = np.arange(128)[:, None]
    t_idx = np.arange(TCH)[None, :]
    for k in range(SB):
        masks[k] = (128 * k + s_idx <= t_idx).astype(np.float32)
    masks = masks.reshape(SB * 128, TCH)

    # hidden^T in [ki, ko, t] tiling, pre-chunked
    hidT = hidden_states.T  # [HID, T]
    hid_re = hidT.reshape(KS, 128, T).transpose(1, 0, 2)  # [ki, ko, t]
    hid8 = np.ascontiguousarray(
        hid_re.reshape(128, KS, A_NCH, ACH).transpose(2, 0, 1, 3)
    ).astype(np.float32)  # [A_NCH, 128, KS, ACH]

    perm = np.concatenate([np.arange(0, HD, 2), np.arange(1, HD, 2)])
    bf16 = mybir.dt.np(BF16)

    ins = []
    for r in range(N_CORES):
        cols = []
        for h in range(HPC):
            base = (HPC * r + h) * HD
            cols.append(base + perm)
        cols.append(NH * HD + r * HD + perm)                # k head, permuted
        cols.append(NH * HD + NKV * HD + r * HD + np.arange(HD))  # v head
        cols = np.concatenate(cols)
        wp = w_qkv[:, cols]  # [HID, 768]
        wp_re = wp.reshape(KS, 128, QKV_M * 128).transpose(1, 0, 2)  # ki ko c
        wq6 = np.ascontiguousarray(
            wp_re.reshape(128, KS, QKV_M, 128).transpose(2, 0, 1, 3)
        ).astype(np.float32)

        wo_sh = w_o[:, 512 * r:512 * (r + 1)]  # [HID, 512]
        wo_re = np.ascontiguousarray(
            wo_sh.reshape(KS, 128, 512).transpose(1, 0, 2)
        ).astype(bf16)

        ins.append({
            "hid8": hid8, "wq6": wq6, "c1": c1, "c2": c2,
            "masks": masks, "wo": wo_re,
        })
    return ins


_PROGRAM = None


def kernel(positions, hidden_states, w_qkv, w_o):
    global _PROGRAM
    if _PROGRAM is None:
        _PROGRAM = build_program()
    nc = _PROGRAM
    ins = make_core_inputs(positions, hidden_states, w_qkv, w_o)
    res = run_bass_kernel_spmd(nc, ins, list(range(N_CORES)))
    out = np.empty((T, HID), dtype=np.float32)
    for r in range(N_CORES):
        out[:, 512 * r:512 * (r + 1)] = res.results[r]["outT"].T
    return out


# revision 7
# speedup vs baseline: 2.0694x; 2.0694x over previous
"""Ernie4.5 attention block (T=2048, H=4096, 32 Q heads / 8 KV heads, rope,
causal, o_proj) on 8 Trainium2 NeuronCores.

Sharding: tensor-parallel by head. Each core computes QKV^T for its 4 Q heads
+ 1 KV head (column-sharded w_qkv), runs attention for those heads, then the
attention outputs (kept transposed, head-dim on partitions) are AllGather'd
and each core computes a 512-column shard of o_proj (column-sharded w_o).
The host only does layout work: transpose/permute/shard inputs, concatenate
output shards.

Device numerics: matmuls in float32r (QKV, QK^T, PV, Z) and bfloat16
(o_proj); softmax in fp32 without max-subtraction (scores are O(6), exp is
safe in fp32); denominator Z computed on the PE with a ones-vector matmul and
divided out after PV.
"""

import numpy as np

import concourse.bacc as bacc
import concourse.mybir as mybir
import concourse.tile as tile
from concourse.bass_utils import run_bass_kernel_spmd
from concourse.masks import make_identity

T = 2048
HID = 4096
NH = 32
NKV = 8
HD = 128
THETA = 500000.0
SCALE = HD ** -0.5
N_CORES = 8
HPC = NH // N_CORES          # q heads per core
KS = HID // 128              # 32 k-subtiles
ACH = 256                    # QKV-phase t-chunk width
A_NCH = T // ACH
TCH = 512                    # attention t-chunk width
NTCH = T // TCH
SB = TCH // 128              # s-tiles per t-chunk (4)
OCH = 256                    # o_proj t-subchunk width

F32 = mybir.dt.float32
F32R = mybir.dt.float32r
BF16 = mybir.dt.bfloat16

QKV_M = HPC + 2              # m-tiles in qkv^T: 4 q heads, k, v
KROW = HPC * 128             # row offset of k block in qkvT (512)
VROW = KROW + 128            # row offset of v block (640)


def _emit_body(nc, tc, io, rep):
    hid8, wq6, c1_d, c2_d, masks_d, wo_d, outT_d = io

    with (
        tc.tile_pool(name=f"const{rep}", bufs=1) as cpool,
        tc.tile_pool(name=f"dram{rep}", bufs=1, space="DRAM") as dpool,
    ):
        # ---- constants ----
        c1 = cpool.tile([128, T], F32)
        c2 = cpool.tile([128, T], F32)
        nc.sync.dma_start(c1[:], c1_d[:])
        nc.sync.dma_start(c2[:], c2_d[:])
        masks = cpool.tile([128, SB, TCH], F32)
        nc.sync.dma_start(masks[:], masks_d.rearrange("(k p) t -> p k t", p=128))
        ident = cpool.tile([128, 128], F32)
        make_identity(nc, ident[:])
        ones_f = cpool.tile([128, 1], F32)
        nc.vector.memset(ones_f[:], 1.0)
        ones = cpool.tile([128, 1], F32R)
        nc.vector.tensor_copy(ones[:], ones_f[:])

        qkvT_dram = dpool.tile([QKV_M * 128, T], F32)

        # ---- phase A: qkvT[c, t] = w_qkvp.T @ hidden.T (column-sharded) ----
        with (
            tc.tile_pool(name=f"pa{rep}", bufs=1) as pa,
            tc.tile_pool(name=f"psa{rep}", bufs=4, space="PSUM") as psa,
        ):
            wt = []
            for m in range(QKV_M):
                w = pa.tile([128, KS, 128], F32R, tag=f"wt{m}")
                nc.sync.dma_start(w[:], wq6[m].bitcast(F32R))
                wt.append(w)
            def load_hid(ci):
                # prefetched a chunk ahead so this DMA issues on the SP queue
                # before the previous chunk's writebacks (which carry waits)
                ht = pa.tile([128, KS, ACH], F32R, tag="hid", bufs=2)
                nc.sync.dma_start(ht[:], hid8[ci].bitcast(F32R))
                return ht

            nxt = load_hid(0)
            for ci in range(A_NCH):
                ht = nxt
                if ci + 1 < A_NCH:
                    nxt = load_hid(ci + 1)
                for m in range(QKV_M):
                    ps = psa.tile([128, ACH], F32, tag="psA")
                    for k in range(KS):
                        nc.tensor.matmul(
                            ps[:], wt[m][:, k], ht[:, k],
                            start=(k == 0), stop=(k == KS - 1),
                        )
                    ot = pa.tile([128, ACH], F32, tag="aout", bufs=3)
                    nc.vector.tensor_copy(ot[:], ps[:])
                    nc.sync.dma_start(
                        qkvT_dram[m * 128:(m + 1) * 128, ci * ACH:(ci + 1) * ACH],
                        ot[:],
                    )

        # ---- phase B: rope, attention, allgather, o_proj ----
        with tc.tile_pool(name=f"pb{rep}", bufs=1) as pb:
            wot = pb.tile([128, KS, 512], BF16)
            nc.sync.dma_start(wot[:], wo_d[:])

            def rope(dst, src_rows):
                # dst is F32R; only the final add writes it (f32r rounding).
                raw = pb.tile([128, T], F32, tag="rraw", bufs=2)
                nc.sync.dma_start(raw[:], qkvT_dram[src_rows:src_rows + 128, :])
                sw = pb.tile([128, T], F32, tag="rswap", bufs=2)
                nc.vector.tensor_copy(sw[0:64], raw[64:128])
                nc.vector.tensor_copy(sw[64:128], raw[0:64])
                ta = pb.tile([128, T], F32, tag="ropetmp", bufs=2)
                nc.vector.tensor_tensor(ta[:], raw[:], c1[:], mybir.AluOpType.mult)
                nc.vector.tensor_tensor(sw[:], sw[:], c2[:], mybir.AluOpType.mult)
                nc.vector.tensor_tensor(dst[:], ta[:], sw[:], mybir.AluOpType.add)

            kT = pb.tile([128, T], F32R, tag="kT")
            rope(kT, KROW)

            # v: transpose [HD, T] -> 16 tiles of [s,128d]
            vraw = pb.tile([128, T], F32, tag="rraw", bufs=2)
            nc.sync.dma_start(vraw[:], qkvT_dram[VROW:VROW + 128, :])
            v_nat = pb.tile([128, T // 128, 128], F32R, tag="vnat")
            with tc.tile_pool(name=f"pst{rep}", bufs=2, space="PSUM") as pst:
                for j in range(T // 128):
                    tp = pst.tile([128, 128], F32, tag="tp")
                    nc.tensor.transpose(tp[:], vraw[:, j * 128:(j + 1) * 128], ident[:])
                    nc.vector.tensor_copy(v_nat[:, j], tp[:])

            qT = []
            for h in range(HPC):
                q = pb.tile([128, T], F32R, tag=f"qT{h}")
                rope(q, h * 128)
                qT.append(q)

            with tc.tile_pool(name=f"psb{rep}", bufs=1, space="PSUM") as psb:
                for c in range(NTCH):
                    ag_in = dpool.tile([HPC * 128, TCH], BF16, tag="agin", bufs=2)
                    nj = SB * (c + 1)
                    for h in range(HPC):
                        pv = psb.tile([128, TCH], F32, tag="pv", bufs=2)
                        zp = psb.tile([1, TCH], F32, tag="z", bufs=1)
                        for j in range(nj):
                            sc = psb.tile([128, TCH], F32, tag="sc", bufs=2)
                            nc.tensor.matmul(
                                sc[:],
                                kT[:, j * 128:(j + 1) * 128],
                                qT[h][:, c * TCH:(c + 1) * TCH],
                                start=True, stop=True,
                            )
                            e = pb.tile([128, TCH], F32R, tag="expT", bufs=3)
                            kd = j - SB * c
                            if kd >= 0:
                                # diagonal block: exp to fp32 then mask-mult
                                # (the mult rounds to f32r)
                                e0 = pb.tile([128, TCH], F32, tag="expTmp", bufs=2)
                                nc.scalar.activation(
                                    e0[:], sc[:], mybir.ActivationFunctionType.Exp,
                                    scale=SCALE,
                                )
                                nc.vector.tensor_tensor(
                                    e[:], e0[:], masks[:, kd], mybir.AluOpType.mult
                                )
                            else:
                                nc.scalar.activation(
                                    e[:], sc[:], mybir.ActivationFunctionType.Exp,
                                    scale=SCALE,
                                )
                            nc.tensor.matmul(
                                pv[:], v_nat[:, j], e[:],
                                start=(j == 0), stop=(j == nj - 1),
                            )
                            nc.tensor.matmul(
                                zp[:], ones[:], e[:],
                                start=(j == 0), stop=(j == nj - 1),
                            )
                        zr = pb.tile([1, TCH], F32, tag="zr", bufs=2)
                        nc.vector.reciprocal(zr[:], zp[:])
                        zb = pb.tile([128, TCH], F32, tag="zb", bufs=2)
                        nc.gpsimd.partition_broadcast(zb[:], zr[:])
                        at = pb.tile([128, TCH], BF16, tag="attnT", bufs=2)
                        nc.vector.tensor_tensor(
                            at[:], pv[:], zb[:], mybir.AluOpType.mult
                        )
                        nc.sync.dma_start(ag_in[h * 128:(h + 1) * 128, :], at[:])

                    ag_out = dpool.tile(
                        [NH * HD, TCH], BF16, tag="agout", bufs=2, addr_space="Shared"
                    )
                    nc.gpsimd.collective_compute(
                        "AllGather",
                        mybir.AluOpType.bypass,
                        replica_groups=[list(range(N_CORES))],
                        ins=[ag_in[:].opt()],
                        outs=[ag_out[:].opt()],
                    )

                    # o_proj for this t-chunk: outT[n, t] += w_o[:,n].T @ attnT
                    ag_re = ag_out.rearrange("(ko ki) t -> ki ko t", ki=128)
                    for s in range(TCH // OCH):
                        rt = pb.tile([128, KS, OCH], BF16, tag="agsb", bufs=2)
                        nc.sync.dma_start(
                            rt[:], ag_re[:, :, s * OCH:(s + 1) * OCH]
                        )
                        for m in range(4):
                            po = psb.tile([128, OCH], F32, tag="po", bufs=2)
                            for k in range(KS):
                                nc.tensor.matmul(
                                    po[:], wot[:, k, m * 128:(m + 1) * 128], rt[:, k],
                                    start=(k == 0), stop=(k == KS - 1),
                                )
                            oo = pb.tile([128, OCH], F32, tag="oout", bufs=2)
                            nc.vector.tensor_copy(oo[:], po[:])
                            nc.sync.dma_start(
                                outT_d[m * 128:(m + 1) * 128,
                                       c * TCH + s * OCH:c * TCH + (s + 1) * OCH],
                                oo[:],
                            )


def build_program(reps=1):
    nc = bacc.Bacc("TRN2", target_bir_lowering=False, debug=False,
                   num_devices=N_CORES)
    hid8 = nc.dram_tensor("hid8", [A_NCH, 128, KS, ACH], F32, kind="ExternalInput")
    wq6 = nc.dram_tensor("wq6", [QKV_M, 128, KS, 128], F32, kind="ExternalInput")
    c1_d = nc.dram_tensor("c1", [128, T], F32, kind="ExternalInput")
    c2_d = nc.dram_tensor("c2", [128, T], F32, kind="ExternalInput")
    masks_d = nc.dram_tensor("masks", [SB * 128, TCH], F32, kind="ExternalInput")
    wo_d = nc.dram_tensor("wo", [128, KS, 512], BF16, kind="ExternalInput")
    outT_d = nc.dram_tensor("outT", [512, T], F32, kind="ExternalOutput")
    io = (hid8, wq6, c1_d, c2_d, masks_d, wo_d, outT_d)
    with tile.TileContext(nc) as tc:
        for rep in range(reps):
            _emit_body(nc, tc, io, rep)
    nc.compile()
    return nc


def make_core_inputs(positions, hidden_states, w_qkv, w_o):
    """Host-side layout prep. Returns list of per-core input dicts."""
    positions = np.asarray(positions)
    hidden_states = np.asarray(hidden_states, dtype=np.float32)
    w_qkv = np.asarray(w_qkv, dtype=np.float32)
    w_o = np.asarray(w_o, dtype=np.float32)
    assert np.all(np.diff(positions.astype(np.int64)) > 0), (
        "kernel assumes strictly increasing positions (causal mask == index mask)"
    )

    # rope tables
    half = HD // 2
    inv_freq = 1.0 / (THETA ** (np.arange(0, half, dtype=np.float32) * 2.0 / HD))
    ang = positions.astype(np.float32)[:, None] * inv_freq[None, :]  # [T, 64]
    cosT = np.cos(ang).T.astype(np.float32)  # [64, T]
    sinT = np.sin(ang).T.astype(np.float32)
    c1 = np.ascontiguousarray(np.concatenate([cosT, cosT], axis=0))
    c2 = np.ascontiguousarray(np.concatenate([-sinT, sinT], axis=0))

    # diagonal-block causal masks
    masks = np.zeros((SB, 128, TCH), dtype=np.float32)
    s_idx = np.arange(128)[:, None]
    t_idx = np.arange(TCH)[None, :]
    for k in range(SB):
        masks[k] = (128 * k + s_idx <= t_idx).astype(np.float32)
    masks = masks.reshape(SB * 128, TCH)

    # hidden^T in [ki, ko, t] tiling, pre-chunked
    hidT = hidden_states.T  # [HID, T]
    hid_re = hidT.reshape(KS, 128, T).transpose(1, 0, 2)  # [ki, ko, t]
    hid8 = np.ascontiguousarray(
        hid_re.reshape(128, KS, A_NCH, ACH).transpose(2, 0, 1, 3)
    ).astype(np.float32)  # [A_NCH, 128, KS, ACH]

    perm = np.concatenate([np.arange(0, HD, 2), np.arange(1, HD, 2)])
    bf16 = mybir.dt.np(BF16)

    ins = []
    for r in range(N_CORES):
        cols = []
        for h in range(HPC):
            base = (HPC * r + h) * HD
            cols.append(base + perm)
        cols.append(NH * HD + r * HD + perm)                # k head, permuted
        cols.append(NH * HD + NKV * HD + r * HD + np.arange(HD))  # v head
        cols = np.concatenate(cols)
        wp = w_qkv[:, cols]  # [HID, 768]
        wp_re = wp.reshape(KS, 128, QKV_M * 128).transpose(1, 0, 2)  # ki ko c
        wq6 = np.ascontiguousarray(
            wp_re.reshape(128, KS, QKV_M, 128).transpose(2, 0, 1, 3)
        ).astype(np.float32)

        wo_sh = w_o[:, 512 * r:512 * (r + 1)]  # [HID, 512]
        wo_re = np.ascontiguousarray(
            wo_sh.reshape(KS, 128, 512).transpose(1, 0, 2)
        ).astype(bf16)

        ins.append({
            "hid8": hid8, "wq6": wq6, "c1": c1, "c2": c2,
            "masks": masks, "wo": wo_re,
        })
    return ins


_PROGRAM = None


def kernel(positions, hidden_states, w_qkv, w_o):
    global _PROGRAM
    if _PROGRAM is None:
        _PROGRAM = build_program()
    nc = _PROGRAM
    ins = make_core_inputs(positions, hidden_states, w_qkv, w_o)
    res = run_bass_kernel_spmd(nc, ins, list(range(N_CORES)))
    out = np.empty((T, HID), dtype=np.float32)
    for r in range(N_CORES):
        out[:, 512 * r:512 * (r + 1)] = res.results[r]["outT"].T
    return out
